# revision 1
# baseline (speedup 1.0000x reference)
import numpy as np
import jax
import jax.numpy as jnp

# nn_DigitCaps dynamic routing: B=512, N=1152, O=10 out-capsules, D=16, I=8.
# Sharding: pure data parallel — batch split 64 per core across 8 NeuronCores,
# W (~5.9 MB) replicated; routing is per-sample so no cross-device comms.
#
# Math: with b_ij initialized to 0 and updated as b_ij += u_hat * sum_d(v),
# the logits stay a rank-1 product b_ij = u_hat[b,n,od] * T[b,o] where T
# accumulates sum_d(v) over iterations. Each routing iteration then only
# needs den = sum_n exp(u*T) and num = sum_n u*exp(u*T), i.e. three fused
# passes over u_hat instead of the reference's softmax/broadcast chain.
B, N, O, I, D = 512, 1152, 10, 8, 16
N_CORES = 8


def _routing_shard(x, W):
    bl = x.shape[0]
    u = jnp.einsum('bni,nodi->bnod', x, W)          # (bl, N, O, D)

    def squash_factor(s):
        sn = jnp.sum(s * s, axis=-1, keepdims=True)  # (bl,O,1)
        return jnp.sqrt(sn) / (1.0 + sn)             # sn*s/((1+sn)*sqrt(sn)) == s*sqrt(sn)/(1+sn)

    # iter 1: softmax(0) is uniform -> s = mean over n
    s = jnp.mean(u, axis=1)                          # (bl, O, D)
    umax = jnp.max(u, axis=1)                        # (bl, O, D) — for stable exp
    umin = jnp.min(u, axis=1)
    f = squash_factor(s)
    T = jnp.sum(f * s, axis=-1, keepdims=True)       # (bl, O, 1) = sum_d v
    for _ in range(2):                               # iters 2 and 3
        tl = T[:, None, :, :]                        # (bl,1,O,1)
        m = jnp.maximum(umax * T, umin * T)          # (bl,O,D) = max_n(u*T)
        e = jnp.exp(u * tl - m[:, None, :, :])
        den = jnp.sum(e, axis=1)                     # (bl, O, D)
        num = jnp.sum(e * u, axis=1)                 # (bl, O, D)
        s = num / den
        f = squash_factor(s)
        T = T + jnp.sum(f * s, axis=-1, keepdims=True)
    return f * s                                     # v = squash(s)  (bl, O, D)


_pmapped = jax.pmap(_routing_shard, in_axes=(0, 0), devices=jax.devices()[:N_CORES])
_W_cache = {}  # id/fingerprint -> replicated device buffers (W is reused across calls)


def _replicated_W(W: np.ndarray):
    key = (W.shape, W.dtype.str, hash(W[::97, 0, 0, 0].tobytes()))
    if key not in _W_cache:
        _W_cache.clear()
        _W_cache[key] = jax.device_put_replicated(W, jax.devices()[:N_CORES])
    return _W_cache[key]


def kernel(x: np.ndarray, W: np.ndarray) -> np.ndarray:
    x = np.ascontiguousarray(x, dtype=np.float32)
    W = np.ascontiguousarray(W, dtype=np.float32)
    xs = x.reshape(N_CORES, B // N_CORES, N, I)
    out = _pmapped(xs, _replicated_W(W))  # (8, 64, O, D)
    return np.asarray(out).reshape(B, O, D)



# revision 14
# speedup vs baseline: 1.1444x; 1.1444x over previous
import numpy as np

# nn_DigitCaps dynamic routing on TRN2 Bass: B=512, N=1152, O=10, D=16, I=8.
#
# Math: with b_ij zero-init and updated as b_ij += u_hat * sum_d(v), the
# routing logits stay rank-1: b = u_hat[b,n,o,d] * T[b,o] with T accumulating
# sum_d(v) across iterations.  Each iteration needs, per (b,o,d):
#   den = sum_n exp(u*T - m),  num = sum_n u*exp(u*T - m),  s = num/den,
# then v = squash(s), T += sum_d v.  Routing is independent per (b,o) pair.
#
# Sharding: 8 cores = 4 batch-groups x 2 o-groups (no collectives).  Each core
# owns 128 batch samples x 5 output capsules, processed as 5 chunks (one per
# o) with SBUF partition dim = the 128 batch samples.  T and the stability
# bias are then per-partition scalars, so exp folds into one ACT op per tile.
#
# u_hat is computed once per chunk by TensorE: stationary = x packed as
# [(16n x 8i)=128, 128b] blocks (one LDW per 16 n's), moving = W packed
# block-diagonally [(16n x 8i)=128, (16n x 16d)=256], so each matmul yields
# u for 16 n x 16 d x 128 b with N=256 streamed columns (all useful).
#
# Stability: m = |T| * max_n(||W[n,o,d,:]|| * ||x[b,n,:]||) - SHIFT upper-
# bounds max_n(u*T) via Cauchy-Schwarz (host precomputes the norm products),
# so no on-device max/min passes are needed.  s = num/den is exact under any
# per-(b,o) shift; SHIFT=60 keeps both ends of exp inside fp32 range.

B, N, O, I, D = 512, 1152, 10, 8, 16
N_CORES = 8
BG, OG = 4, 2            # core grid: batch-groups x o-groups
BL = B // BG             # 128 batch per core
OL = O // OG             # 5 o-capsules per core
NBLK = N // 16           # 72 blocks of 16 input capsules
SHIFT = 60.0

_STATE = {}


def _build_bass(stage=4, sub=9):
    import concourse.bass as bass
    import concourse.tile as tile
    from concourse import bacc, mybir

    f32 = mybir.dt.float32
    AX = mybir.AxisListType
    OP = mybir.AluOpType
    AF = mybir.ActivationFunctionType

    nc = bacc.Bacc("TRN2", target_bir_lowering=False, debug=False,
                   num_devices=N_CORES)

    xs_d = nc.dram_tensor("xs", [128, NBLK, BL], f32, kind="ExternalInput").ap()
    wbd_d = nc.dram_tensor("wbd", [OL, 128, NBLK, 256], f32, kind="ExternalInput").ap()
    wd_d = nc.dram_tensor("wd", [OL, 128, NBLK, 16], f32, kind="ExternalInput").ap()
    kcs_d = nc.dram_tensor("kcs", [OL, BL, 1], f32, kind="ExternalInput").ap()
    v_d = nc.dram_tensor("v", [OL, BL, 16], f32, kind="ExternalOutput").ap()

    with tile.TileContext(nc) as tc:
        from contextlib import ExitStack
        ctx = ExitStack()
        u_pool = ctx.enter_context(tc.tile_pool(name="u", bufs=1))
        wbd_pool = ctx.enter_context(tc.tile_pool(name="wbd", bufs=1))
        wd_pool = ctx.enter_context(tc.tile_pool(name="wd", bufs=1))
        xs_pool = ctx.enter_context(tc.tile_pool(name="xs", bufs=6))
        e_pool = ctx.enter_context(tc.tile_pool(name="e", bufs=2))
        st_pool = ctx.enter_context(tc.tile_pool(name="st", bufs=1))
        pu_pool = ctx.enter_context(tc.tile_pool(name="pu", bufs=3, space="PSUM"))
        ps1_pool = ctx.enter_context(tc.tile_pool(name="ps1", bufs=1, space="PSUM"))
        pd_pool = ctx.enter_context(tc.tile_pool(name="pd", bufs=2, space="PSUM"))

        shift_t = st_pool.tile([BL, 1], f32, name="shift")
        nc.vector.memset(shift_t[:], SHIFT)

        for o in range(OL):
            wbd_t = wbd_pool.tile([128, NBLK, 256], f32, name=f"wbd{o}", tag="wbd")
            nc.sync.dma_start(wbd_t[:], wbd_d[o])
            wd_t = wd_pool.tile([128, NBLK, 16], f32, name=f"wd{o}", tag="wd")
            nc.sync.dma_start(wd_t[:], wd_d[o])
            kcs_t = st_pool.tile([BL, 1], f32, name=f"kcs{o}", tag="kcs")
            nc.sync.dma_start(kcs_t[:], kcs_d[o])

            u_t = u_pool.tile([128, D, N], f32, name=f"u{o}", tag="u")
            ps1 = ps1_pool.tile([128, 16], f32, name=f"s1p{o}", tag="s1p")

            # ---- phase 1: u_hat matmuls + PSUM->SBUF scatter copies ----
            for bp in range(NBLK // 2):
                pu = pu_pool.tile([128, 2, 256], f32, name=f"pu{o}_{bp}", tag="pu")
                for j in (0, 1):
                    blk = 2 * bp + j
                    xs_t = xs_pool.tile([128, BL], f32, name=f"x{o}_{blk}", tag="xs")
                    nc.sync.dma_start(xs_t[:], xs_d[:, blk, :])
                    nc.tensor.matmul(pu[:, j, :], lhsT=xs_t[:],
                                     rhs=wbd_t[:, blk, :],
                                     start=True, stop=True,
                                     skip_group_check=True)
                    nc.tensor.matmul(ps1[:], lhsT=xs_t[:], rhs=wd_t[:, blk, :],
                                     start=(blk == 0), stop=(blk == NBLK - 1),
                                     skip_group_check=True)
                # copy [128, (j2, nn16, d16)] -> u[128, d, n=bp*32+j*16+nn]
                src = pu[:].rearrange("p j (nn d) -> p j nn d", nn=16, d=16)
                dst = u_t[:, :, bp * 32:(bp + 1) * 32].rearrange(
                    "p d (j nn) -> p j nn d", j=2, nn=16)
                if bp % 2 == 0:
                    nc.vector.tensor_copy(dst, src)
                else:
                    nc.scalar.copy(dst, src)

            # ---- iteration 1 (uniform softmax): s = mean_n u  ----
            s_t = st_pool.tile([BL, 16], f32, name=f"s{o}", tag="s")
            nc.scalar.mul(s_t[:], ps1[:], 1.0 / N)

            sq2 = st_pool.tile([BL, 16], f32, name=f"sq2_{o}", tag="sq2")
            T_t = st_pool.tile([BL, 1], f32, name=f"T{o}", tag="T")
            vout = st_pool.tile([BL, 16], f32, name=f"vo{o}", tag="vo")

            def squash_T(tag, accum_into_T):
                # f = sqrt(sn)/(1+sn); vout = f*s; Tdelta = sum_d vout
                sn = st_pool.tile([BL, 1], f32, name=f"sn{tag}", tag="sn")
                nc.scalar.activation(sq2[:], s_t[:], AF.Square,
                                     accum_out=sn[:])
                if sub < 2:
                    nc.vector.tensor_copy(vout[:], sq2[:])
                    nc.vector.memset(T_t[:], 0.0)
                    return
                lsn = st_pool.tile([BL, 1], f32, name=f"ls{tag}", tag="lsn")
                nc.scalar.activation(lsn[:], sn[:], AF.Ln)
                sqr = st_pool.tile([BL, 1], f32, name=f"sr{tag}", tag="sqr")
                nc.scalar.activation(sqr[:], lsn[:], AF.Exp, scale=0.5)
                if sub < 3:
                    nc.vector.tensor_copy(vout[:], sq2[:])
                    nc.vector.tensor_copy(T_t[:], sqr[:])
                    return
                snp = st_pool.tile([BL, 1], f32, name=f"sp{tag}", tag="snp")
                nc.vector.tensor_scalar_add(snp[:], sn[:], 1.0)
                rsn = st_pool.tile([BL, 1], f32, name=f"rs{tag}", tag="rsn")
                nc.vector.reciprocal(rsn[:], snp[:])
                f_t = st_pool.tile([BL, 1], f32, name=f"f{tag}", tag="f")
                nc.vector.tensor_mul(f_t[:], sqr[:], rsn[:])
                if sub < 4:
                    nc.vector.tensor_copy(vout[:], sq2[:])
                    nc.vector.tensor_copy(T_t[:], f_t[:])
                    return
                dT = st_pool.tile([BL, 1], f32, name=f"dT{tag}", tag="dT")
                nc.scalar.activation(vout[:], s_t[:], AF.Copy, scale=f_t[:],
                                     accum_out=dT[:])
                if accum_into_T:
                    nc.vector.tensor_add(T_t[:], T_t[:], dT[:])
                else:
                    nc.vector.tensor_copy(T_t[:], dT[:])

            if stage < 2:
                nc.sync.dma_start(v_d[o], s_t[:])
                continue
            squash_T(f"{o}i1", accum_into_T=False)
            if stage < 3:
                nc.sync.dma_start(v_d[o], vout[:])
                continue

            den_t = st_pool.tile([BL, 16], f32, name=f"den{o}", tag="den")
            nump = st_pool.tile([BL, 64], f32, name=f"np{o}", tag="np")
            num_t = st_pool.tile([BL, 16], f32, name=f"num{o}", tag="num")

            for it in ((2,) if stage < 4 else (2, 3)):
                negT = st_pool.tile([BL, 1], f32, name=f"gT{o}_{it}", tag="gT")
                nc.vector.tensor_scalar_mul(negT[:], T_t[:], -1.0)
                absT = st_pool.tile([BL, 1], f32, name=f"aT{o}_{it}", tag="aT")
                nc.vector.tensor_max(absT[:], T_t[:], negT[:])
                negm = st_pool.tile([BL, 1], f32, name=f"nm{o}_{it}", tag="nm")
                nc.vector.scalar_tensor_tensor(
                    negm[:], absT[:], kcs_t[:], shift_t[:],
                    op0=OP.mult, op1=OP.add)
                for q8 in range(8):
                    e_t = e_pool.tile([128, 2, N], f32, name=f"e{o}_{it}_{q8}",
                                      tag="e")
                    nc.scalar.activation(e_t[:], u_t[:, 2 * q8:2 * q8 + 2, :],
                                         AF.Exp, bias=negm[:], scale=T_t[:])
                    nc.vector.tensor_reduce(
                        den_t[:, 2 * q8:2 * q8 + 2], e_t[:], axis=AX.X,
                        op=OP.add)
                    for dd in range(2):
                        d = 2 * q8 + dd
                        for q in range(4):
                            pd = pd_pool.tile([128, 288], f32,
                                              name=f"pd{o}_{it}_{d}_{q}",
                                              tag="pd")
                            nc.vector.scalar_tensor_tensor(
                                out=pd[:],
                                in0=e_t[:, dd, q * 288:(q + 1) * 288],
                                scalar=1.0,
                                in1=u_t[:, d, q * 288:(q + 1) * 288],
                                op0=OP.mult, op1=OP.mult,
                                accum_out=nump[:, d * 4 + q:d * 4 + q + 1])
                nc.vector.tensor_reduce(
                    num_t[:], nump[:].rearrange("p (d q) -> p d q", q=4),
                    axis=AX.X, op=OP.add)
                rden = st_pool.tile([BL, 16], f32, name=f"rd{o}_{it}", tag="rd")
                nc.vector.reciprocal(rden[:], den_t[:])
                nc.vector.tensor_mul(s_t[:], num_t[:], rden[:])
                squash_T(f"{o}i{it}", accum_into_T=(it == 2))

            nc.sync.dma_start(v_d[o], vout[:])
        ctx.close()

    nc.compile()
    return nc


def _pack_w(W):
    # W: [N, O, D, I] fp32.  Per o-group: block-diagonal moving operand
    # wbd[o, p=(nn,i), blk, c=(nn2,d)] plus dense wd for the iter-1 mean.
    packs = []
    for og in range(OG):
        Wl = W[:, og * OL:(og + 1) * OL]                     # [N, 5, D, I]
        Wt = Wl.reshape(NBLK, 16, OL, D, I)                  # [blk, nn, o, d, i]
        bd = np.zeros((OL, 16, I, NBLK, 16, D), np.float32)  # [o,nn,i,blk,nn2,d]
        r = np.arange(16)
        bd[:, r, :, :, r, :] = Wt.transpose(1, 2, 4, 0, 3)   # [nn,o,i,blk,d]
        wbd = np.ascontiguousarray(
            bd.reshape(OL, 128, NBLK, 256))                  # [o, p, blk, c]
        wd = np.ascontiguousarray(
            Wt.transpose(2, 1, 4, 0, 3).reshape(OL, 128, NBLK, 16))
        wmax = np.sqrt((Wl ** 2).sum(-1)).max(axis=(0, 2))   # [5]
        packs.append((wbd, wd, wmax.astype(np.float32)))
    return packs


def _get_state(W):
    key = (W.shape, hash(W[::131, 0, 0, 0].tobytes()))
    st = _STATE.get("st")
    if st is not None and st["key"] == key:
        return st
    nc = _STATE.get("nc")
    if nc is None:
        nc = _build_bass()
        _STATE["nc"] = nc
    st = {"key": key, "nc": nc, "wpacks": _pack_w(W)}
    _STATE["st"] = st
    return st


def _core_inputs(st, x, xnorm):
    # returns in_maps for the 8 cores; core index = og * BG + bg
    in_maps = []
    for og in range(OG):
        wbd, wd, wmax = st["wpacks"][og]
        for bg in range(BG):
            b0 = bg * BL
            xc = x[b0:b0 + BL]                               # [128, N, I]
            xs = np.ascontiguousarray(
                xc.transpose(1, 2, 0).reshape(NBLK, 128, BL)
                .transpose(1, 0, 2))                         # [p, blk, b]
            kcs = -(wmax[:, None] * xnorm[None, b0:b0 + BL])  # [5, 128]
            in_maps.append({
                "xs": xs, "wbd": wbd, "wd": wd,
                "kcs": np.ascontiguousarray(
                    kcs[:, :, None].astype(np.float32)),
            })
    # order in_maps by core id: we used og-major; keep a parallel index list
    return in_maps


class _Runner:
    """Cached PJRT executor for the compiled Bass program (modeled on
    bass2jax.run_bass_via_pjrt, but holding the jitted callable so repeat
    calls don't re-trace)."""

    def __init__(self, nc):
        import jax
        import numpy as _np
        from jax.sharding import Mesh, PartitionSpec
        from jax.experimental.shard_map import shard_map
        from concourse import mybir
        from concourse.bass2jax import (_bass_exec_p, install_neuronx_cc_hook,
                                        partition_id_tensor)

        install_neuronx_cc_hook()
        self.jax = jax
        part_name = (nc.partition_id_tensor.name
                     if nc.partition_id_tensor else None)
        in_names, out_names, out_avals, zero_outs = [], [], [], []
        for alloc in nc.m.functions[0].allocations:
            if not isinstance(alloc, mybir.MemoryLocationSet):
                continue
            name = alloc.memorylocations[0].name
            if alloc.kind == "ExternalInput":
                if name == part_name:
                    continue
                in_names.append(name)
            elif alloc.kind == "ExternalOutput":
                shape = tuple(alloc.tensor_shape)
                dtype = mybir.dt.np(alloc.dtype)
                out_names.append(name)
                out_avals.append(jax.core.ShapedArray(shape, dtype))
                zero_outs.append(_np.zeros(shape, dtype))
        self.in_names, self.out_names = in_names, out_names
        self.out_avals, self.zero_outs = out_avals, zero_outs
        n_params, n_outs = len(in_names), len(out_names)
        all_names = in_names + out_names
        if part_name is not None:
            all_names = all_names + [part_name]

        def _body(*args):
            operands = list(args)
            if part_name is not None:
                operands.append(partition_id_tensor())
            return tuple(_bass_exec_p.bind(
                *operands, out_avals=tuple(out_avals), in_names=tuple(all_names),
                out_names=tuple(out_names), lowering_input_output_aliases=(),
                sim_require_finite=True, sim_require_nnan=True, nc=nc))

        devices = jax.devices()[:N_CORES]
        self.mesh = Mesh(_np.asarray(devices), ("core",))
        in_specs = (PartitionSpec("core"),) * (n_params + n_outs)
        out_specs = (PartitionSpec("core"),) * n_outs
        self.fn = jax.jit(
            shard_map(_body, mesh=self.mesh, in_specs=in_specs,
                      out_specs=out_specs, check_rep=False),
            donate_argnums=tuple(range(n_params, n_params + n_outs)),
            keep_unused=True)

    def concat_inputs(self, in_maps):
        import numpy as _np
        return [_np.concatenate([m[name] for m in in_maps], axis=0)
                for name in self.in_names]

    def zeros(self):
        import numpy as _np
        return [_np.zeros((N_CORES * z.shape[0], *z.shape[1:]), z.dtype)
                for z in self.zero_outs]

    def __call__(self, concat_in):
        return self.fn(*concat_in, *self.zeros())


def _run(st, in_maps):
    runner = st.get("runner")
    if runner is None:
        runner = _Runner(st["nc"])
        st["runner"] = runner
    out_arrs = runner(runner.concat_inputs(in_maps))
    av = runner.out_avals[0]
    import numpy as _np
    v_all = _np.asarray(out_arrs[0]).reshape(N_CORES, *av.shape)
    return v_all


def kernel(x: np.ndarray, W: np.ndarray) -> np.ndarray:
    x = np.ascontiguousarray(x, dtype=np.float32)
    W = np.ascontiguousarray(W, dtype=np.float32)
    st = _get_state(W)
    xnorm = np.sqrt((x ** 2).sum(-1)).max(axis=1).astype(np.float32)  # [B]
    in_maps = _core_inputs(st, x, xnorm)
    v_all = _run(st, in_maps)
    out = np.empty((B, O, D), np.float32)
    ci = 0
    for og in range(OG):
        for bg in range(BG):
            v = v_all[ci]                                     # [5, 128, 16]
            out[bg * BL:(bg + 1) * BL, og * OL:(og + 1) * OL, :] = \
                v.transpose(1, 0, 2)
            ci += 1
    return out


def hw_exec_ns(x: np.ndarray, W: np.ndarray, reps: int = 8) -> int:
    """Best-of-reps device-resident execution time of the compiled NEFF."""
    import time
    import jax
    x = np.ascontiguousarray(x, dtype=np.float32)
    W = np.ascontiguousarray(W, dtype=np.float32)
    st = _get_state(W)
    xnorm = np.sqrt((x ** 2).sum(-1)).max(axis=1).astype(np.float32)
    in_maps = _core_inputs(st, x, xnorm)
    runner = st.get("runner")
    if runner is None:
        runner = _Runner(st["nc"])
        st["runner"] = runner
    from jax.sharding import NamedSharding, PartitionSpec
    sh = NamedSharding(runner.mesh, PartitionSpec("core"))
    concat_in = runner.concat_inputs(in_maps)
    dev_in = [jax.device_put(a, sh) for a in concat_in]
    dev_zero_sets = [[jax.device_put(z, sh) for z in runner.zeros()]
                     for _ in range(reps + 1)]
    jax.block_until_ready(runner.fn(*dev_in, *dev_zero_sets[0]))  # warm
    best = float("inf")
    for i in range(reps):
        t0 = time.perf_counter()
        jax.block_until_ready(runner.fn(*dev_in, *dev_zero_sets[i + 1]))
        best = min(best, time.perf_counter() - t0)
    return int(best * 1e9)
